# revision 37
# baseline (speedup 1.0000x reference)
"""Trainium2 Bass kernel for nn_Attention (no-softmax attention block).

Reference computation (per batch):
    q = x @ Wq.T + bq ; k = x @ Wk.T + bk ; v = x @ Wv.T + bv   (H=12 heads, D=64)
    att = (q k^T) / sqrt(D)      (NO softmax)
    y   = att @ v ;  out = y @ Wp.T + bp

Key algebraic optimizations: without softmax, (q k^T) v == q (k^T v),
where k^T v is a [D, D] matrix per (batch, head) — the [T, T] attention
matrix is never materialized. Wp is additionally folded into that small
matrix: out = q_scaled @ N_b + bp, with N_{b,h} = M'_h^T-mm @ WpT_h and
M'_h = Vh^T Kh, so att@v and the separate y@Wp.T pass both disappear
(y is never formed; ~6k fewer streamed PE columns per core).

Sharding: data-parallel over batch (8 cores x 2 batches), no collectives.
Compute: bf16 matmuls, fp32 PSUM accumulation; bf16 biases and bf16
output (host upcasts to f32; all well within the error budget).

Per-core device layout (tokens TOK = 2048):
    xT [C, TOK] (channels on partitions)  ->  QT [C, TOK] (x 1/sqrt(D), +bq)
                                          ->  K, V natural [TOK, C] (+bias)
    M'[b,hpair] = Vh^T @ Kh  [D, D] blocks (PSUM accum over token tiles)
    N[b,hpair] [128, C] = M'-stationary matmuls streaming WpT rows
    out [TOK, C] = qt-token-tile-stationary matmuls against N_b, + bp

Raw-bass engine programs (no Tile): SP does DMA, PE all matmuls, ACT the
transposed-layout PSUM drains (fused scale/bias), DVE the natural-layout
drains (broadcast bias adds); M'/N phases are software-pipelined and the
N drains alternate ACT/DVE. Explicit semaphores; waits are standalone
instructions so no 64B-struct sync-slot limits apply. Startup: a small
"boot" DMA (wq col-block 0 + x chunk 0 + bq) gates the first matmuls
while dummy matmuls warm the PE clock gate (long 512-col streams — short
streams make the DVFS governor latch a 2.0 GHz state for the whole
kernel); wq 1-3 issue pre-barrier on the SP ring and x chunks 1-2 on the
ACT HW-DGE ring, because each DMA_DIRECT2D issue costs ~0.6us of engine
time and post-barrier issues land too late for the QT sweep. Host-side
packs (boot, col-major wq, pre-broadcast bias rows) keep every DMA's
contiguous runs >= 512B and the DMA order aligned with consumption.
"""

import numpy as np
from ml_dtypes import bfloat16

B, T, C, H = 16, 1024, 768, 12
D = C // H                 # 64
N_CORES = 8
BP = B // N_CORES          # batches per core
TOK = BP * T               # tokens per core
CT = C // 128              # 6 channel tiles
TT = TOK // 128            # 16 token tiles
HPAIRS = CT                # 6 head pairs (2 heads per 128-channel tile)
QCH = 512
OCH = 384                  # C split into 2x384 output chunks (1 PSUM bank fp32)
SCALE = 1.0 / float(np.sqrt(D))

_CACHE = {}


def _build_nc():
    import concourse.bass as bass
    from concourse import mybir

    bf16 = mybir.dt.bfloat16
    f32 = mybir.dt.float32
    Ident = mybir.ActivationFunctionType.Identity

    nc = bass.Bass()

    # boot pack: wq cols 0:128 + xT chunk 0 + bq column, partition-major —
    # the minimal prefix that unblocks the first QT group (and its ACT
    # drain), in one DMA
    boot_d = nc.declare_dram_parameter("boot", [128, CT, 128 + QCH + 1], bf16, isOutput=False)
    xT_d = nc.declare_dram_parameter("xT", [C, TOK], bf16, isOutput=False)
    # wq packed col-block-major [p, co, a, 128]: per-co DMAs get 1536B
    # contiguous runs (256B runs pay a 2x DMA latency penalty)
    wq_d = nc.declare_dram_parameter("wqp", [128, CT, CT, 128], bf16, isOutput=False)
    wk_d = nc.declare_dram_parameter("wkT", [C, C], bf16, isOutput=False)
    wv_d = nc.declare_dram_parameter("wvT", [C, C], bf16, isOutput=False)
    wp_d = nc.declare_dram_parameter("wpT", [C, C], bf16, isOutput=False)
    # broadcast-row biases [bk | bv | bp]; bq rides in the boot pack, so
    # this tensor isn't needed until the first K-phase drain and can sit
    # late in the DMA stream
    bias_d = nc.declare_dram_parameter("biases", [128, 3 * C], bf16, isOutput=False)
    # bf16 output (host upcasts): halves the output DMA traffic/tail
    out_d = nc.declare_dram_parameter("out", [TOK, C], bf16, isOutput=True)

    def bcast(dram_handle):
        ap = dram_handle[:]
        return bass.AP(tensor=ap.tensor, offset=ap.offset, ap=[[0, 128]] + list(ap.ap))

    import contextlib
    stack = contextlib.ExitStack()
    sb = lambda name, shape, dt: stack.enter_context(nc.sbuf_tensor(name, shape, dt))
    ps = lambda name, shape, dt: stack.enter_context(nc.psum_tensor(name, shape, dt))
    sem = lambda name: stack.enter_context(nc.semaphore(name))

    with stack:
        boot_sb = sb("boot_sb", [128, CT, 128 + QCH + 1], bf16)
        wq_sb = sb("wq_sb", [128, CT, CT, 128], bf16)
        xt_sb = sb("xt_sb", [128, CT, TOK], bf16)
        wk_sb = sb("wk_sb", [128, CT, C], bf16)
        wv_sb = sb("wv_sb", [128, CT, C], bf16)
        wp_sb = sb("wp_sb", [128, CT, C], bf16)
        qt_sb = sb("qt_sb", [128, CT, TOK], bf16)
        k_sb = sb("k_sb", [128, TT, C], bf16)
        v_sb = sb("v_sb", [128, TT, C], bf16)
        m_sb = sb("m_sb", [128, BP * HPAIRS, 2 * D], bf16)
        n_sb = sb("n_sb", [128, BP * HPAIRS, C], bf16)
        NOT = 4
        ot_sb = [sb(f"ot_sb{i}", [128, C], bf16) for i in range(NOT)]
        bias_sb = sb("bias_sb", [128, 3 * C], bf16)
        bk_bc = bias_sb[:, 0:C]
        bv_bc = bias_sb[:, C:2 * C]
        bp_bc = bias_sb[:, 2 * C:3 * C]

        proj_ps = [ps(f"proj_ps{i}", [128, QCH], f32) for i in range(3)]
        m_ps = [ps(f"m_ps{i}", [128, D], f32) for i in range(2)]
        py_ps = [ps(f"py_ps{i}", [128, QCH], f32) for i in range(3)]

        sem_boot = sem("s_boot")
        sem_wqc = [sem(f"s_wq{i}") for i in range(CT)]
        sem_xch = [sem(f"s_x{i}") for i in range(TOK // QCH)]
        sem_wk, sem_wv, sem_wp, sem_b = (
            sem("s_wk"), sem("s_wv"), sem("s_wp"), sem("s_b"))
        sem_pe, sem_act, sem_dve = sem("s_pe"), sem("s_act"), sem("s_dve")
        sem_out = [sem(f"s_out{i}") for i in range(NOT)]

        # Defensive semaphore zeroing: allocation does not clear sems, and a
        # prior execution of this NEFF leaves them at final values (all waits
        # would pass immediately -> races). Each semaphore is cleared by the
        # engine that increments it, BEFORE that engine's first increment;
        # the barrier then orders clears against every consumer's first wait.
        # SP (and ACT for the biases) additionally issues its input DMAs
        # before joining the barrier — its sems are already cleared, and no
        # consumer can observe them until after the barrier.
        # The boot DMA (all of wq + x chunk 0 -> the whole first QT sweep)
        # is the startup critical path: clear only its semaphore, issue it,
        # then clear the remaining sems while the transfer runs, then
        # barrier. The bias pack rides the scalar engine's separate HW-DGE
        # ring. Everything else is issued post-barrier so the barrier isn't
        # delayed by DMA issue time.
        nc.sync.sem_clear(sem_boot)
        nc.sync.sem_clear(sem_b)
        for s in sem_wqc[1:]:
            nc.sync.sem_clear(s)
        # pre-barrier SP DMA sequence; ring FIFO fixes the device order:
        # boot (first QT group + its bq bias column), then wq per column
        # block (later QT groups). All issued PRE-barrier: each
        # DMA_DIRECT2D issue costs ~0.6us of engine time, and SP would
        # otherwise sit idle at the barrier while PE warms up — issuing
        # here gets wq5's data landed before the QT sweep reaches it.
        nc.sync.dma_start(out=boot_sb[:], in_=boot_d[:]).then_inc(sem_boot, 16)
        for co in (1, 2, 3):
            nc.sync.dma_start(
                out=wq_sb[:, co, :, :], in_=wq_d[:, co, :, :]
            ).then_inc(sem_wqc[co], 16)
        for s in (sem_wk, sem_wv, sem_wp, *sem_out):
            nc.sync.sem_clear(s)
        # x chunks 1-3 ride the ACT engine's separate HW-DGE ring, in
        # parallel with the SP ring (boot + weights): doubles early input
        # bandwidth so the QT phase never starves. ACT clears the sems it
        # increments, then issues, all pre-barrier.
        nc.scalar.sem_clear(sem_act)
        for tch in range(1, TOK // QCH):
            nc.scalar.sem_clear(sem_xch[tch])

        def x_dma(e, tch):
            t0 = tch * QCH
            x_ap = xT_d[:, t0:t0 + QCH].rearrange("(a p) t -> p a t", p=128)
            e.dma_start(out=xt_sb[:, :, t0:t0 + QCH], in_=x_ap
                        ).then_inc(sem_xch[tch], 16)
        x_dma(nc.scalar, 1)
        x_dma(nc.scalar, 2)
        nc.tensor.sem_clear(sem_pe)
        nc.vector.sem_clear(sem_dve)
        # PE warm-up (pre-barrier): ~3.8us of sustained matmuls lifts the
        # HAM clock gate 1.2 -> 2.4 GHz while the boot DMA streams. The
        # streams must be LONG (512 cols): short ones leave LDW bubbles in
        # the duty cycle and the DVFS governor then latches a 2.0 GHz
        # state for the whole kernel (measured: every phase ran exactly
        # 1.2x slower with 128-col warmups). Inputs are garbage SBUF; the
        # scratch PSUM slot's first real use is much later and opens with
        # start=True.
        for _w in range(10):
            nc.tensor.matmul(py_ps[0][:], xt_sb[:, 0, 0:128], xt_sb[:, 1, 0:QCH],
                             start=True, stop=True)

        nc.all_engine_barrier()

        # ---------------- plan ----------------
        ops = {"sp": [], "pe": [], "act": [], "dve": [], "pool": []}
        cnt = {"pe": 0, "act": 0, "dve": 0}
        waited = {k: {} for k in ops}

        # m_sb holds M' as block-diagonal [128, 128] tiles per (b, pair):
        # diag blocks are written by the M' drains; the off-diagonal blocks
        # are zeroed ONCE here (first DVE ops, done ~90us before first use)
        # so the N phase can contract both heads in a single full-array
        # matmul — halving the WpT streaming vs per-head quadrant matmuls.
        ops["dve"].append(lambda e: e.memset(
            m_sb[0:D, :, D:2 * D], 0.0).then_inc(sem_dve))
        ops["dve"].append(lambda e: e.memset(
            m_sb[D:2 * D, :, 0:D], 0.0).then_inc(sem_dve))
        cnt["dve"] += 2

        def emit(eng_key, fn):
            ops[eng_key].append(fn)

        def wait(eng_key, s, thr):
            if thr <= 0:
                return
            if waited[eng_key].get(s.name, 0) < thr:
                waited[eng_key][s.name] = thr
                emit(eng_key, lambda e, s=s, t=thr: e.wait_ge(s, t))

        ENG_SEM = {"act": sem_act, "dve": sem_dve}

        # ---- remaining input DMAs (post-barrier, overlap the QT phase).
        # SP ring order matches consumption: wq4, wq5 (late QT groups),
        # wk (K phase), wv, broadcast biases (first K drain), wp. ACT ring
        # carries x2, x3 in parallel (x1 went pre-barrier). Only the
        # earliest-needed transfers are issued pre-barrier: each issue
        # costs ~0.6us of engine time and would delay the barrier.
        for co in (4, 5):
            emit("sp", lambda e, co=co, s=sem_wqc[co]: e.dma_start(
                out=wq_sb[:, co, :, :], in_=wq_d[:, co, :, :]).then_inc(s, 16))
        emit("act", lambda e: x_dma(e, 3))
        for w_sb, w_d, s in ((wk_sb, wk_d, sem_wk), (wv_sb, wv_d, sem_wv)):
            w_ap = w_d[:].rearrange("(a p) c -> p a c", p=128)
            emit("sp", lambda e, w_sb=w_sb, w_ap=w_ap, s=s: e.dma_start(
                out=w_sb[:], in_=w_ap
            ).then_inc(s, 16))
        emit("sp", lambda e: e.dma_start(
            out=bias_sb[:], in_=bias_d[:]).then_inc(sem_b, 16))
        wp_ap = wp_d[:].rearrange("(a p) c -> p a c", p=128)
        emit("sp", lambda e: e.dma_start(
            out=wp_sb[:], in_=wp_ap
        ).then_inc(sem_wp, 16))

        def xt_slice(ci, t0, n):
            """x-transposed slice; chunk 0 lives in the boot pack."""
            if t0 + n <= QCH:
                return boot_sb[:, ci, 128 + t0:128 + t0 + n]
            return xt_sb[:, ci, t0:t0 + n]

        def wq_slice(ci, co):
            if co == 0:
                return boot_sb[:, ci, 0:128]
            return wq_sb[:, co, ci, :]

        def wait_x(eng, tch):
            wait(eng, sem_boot if tch == 0 else sem_xch[tch], 16)

        def wait_wq(eng, co):
            wait(eng, sem_boot if co == 0 else sem_wqc[co], 16)

        proj_tenant = [None] * 3     # (eng_key, cnt) of last drain of this psum slot
        m_tenant = [None, None]
        py_tenant = [None, None, None]

        def slot_wait(tenants, slot):
            t = tenants[slot]
            if t is not None:
                wait("pe", ENG_SEM[t[0]], t[1])

        # ---- Phase QT: QT[o, t] = wqT^T-mm, scale+bias fused into ACT drain
        qt_drain = {}
        g = 0
        for tch in range(TOK // QCH):
            for co in range(CT):
                t0 = tch * QCH
                slot = g % 3
                pq = proj_ps[slot]
                wait_wq("pe", co)
                wait_x("pe", tch)
                slot_wait(proj_tenant, slot)
                for ci in range(CT):
                    mm = lambda e, ci=ci, co=co, t0=t0, pq=pq: e.matmul(
                        pq[:], wq_slice(ci, co), xt_slice(ci, t0, QCH),
                        start=(ci == 0), stop=(ci == CT - 1))
                    if ci == CT - 1:
                        emit("pe", lambda e, mm=mm: mm(e).then_inc(sem_pe))
                        cnt["pe"] += 1
                    else:
                        emit("pe", mm)
                pe_thr = cnt["pe"]
                wait("act", sem_boot, 16)
                wait("act", sem_pe, pe_thr)
                emit("act", lambda e, co=co, t0=t0, pq=pq: e.activation(
                    out=qt_sb[:, co, t0:t0 + QCH], in_=pq[:], func=Ident,
                    bias=boot_sb[:, co, 128 + QCH:128 + QCH + 1], scale=SCALE
                ).then_inc(sem_act))
                cnt["act"] += 1
                qt_drain[(co, tch)] = cnt["act"]
                proj_tenant[slot] = ("act", cnt["act"])
                g += 1

        # ---- Phases K, V: natural layout [tok, ch], broadcast bias on DVE
        def natural_proj(w_sb, w_sem, dst_sb, bias_bc, drain_dict):
            nonlocal g
            for tt in range(TT):
                for och in range(2):
                    o0 = och * OCH
                    slot = g % 3
                    pv = proj_ps[slot]
                    wait("pe", w_sem, 16)
                    slot_wait(proj_tenant, slot)
                    for ci in range(CT):
                        mm = lambda e, ci=ci, tt=tt, o0=o0, pv=pv, w_sb=w_sb: e.matmul(
                            pv[:, 0:OCH], xt_slice(ci, tt * 128, 128),
                            w_sb[:, ci, o0:o0 + OCH],
                            start=(ci == 0), stop=(ci == CT - 1))
                        if ci == CT - 1:
                            emit("pe", lambda e, mm=mm: mm(e).then_inc(sem_pe))
                            cnt["pe"] += 1
                        else:
                            emit("pe", mm)
                    wait("dve", sem_b, 16)
                    wait("dve", sem_pe, cnt["pe"])
                    emit("dve", lambda e, tt=tt, o0=o0, pv=pv, dst_sb=dst_sb, bias_bc=bias_bc:
                         e.tensor_add(dst_sb[:, tt, o0:o0 + OCH], pv[:, 0:OCH],
                                      bias_bc[:, o0:o0 + OCH]).then_inc(sem_dve))
                    cnt["dve"] += 1
                    drain_dict[(tt, och)] = cnt["dve"]
                    proj_tenant[slot] = ("dve", cnt["dve"])
                    g += 1

        k_drain, v_drain = {}, {}
        natural_proj(wk_sb, sem_wk, k_sb, bk_bc, k_drain)
        natural_proj(wv_sb, sem_wv, v_sb, bv_bc, v_drain)

        # ---- Phases M' and N, software-pipelined: PE computes M'(i) while
        # N(i-1)'s PSUM drains. Without softmax the whole block reduces to
        #   out = q_scaled @ N_b + bp,  N_{b,h} = (V_h^T K_h)^T-mm @ WpT_h
        # so att@v and the separate y@Wp^T pass are never materialized.
        # M'[b,hpair] = Vh^T @ Kh (both heads packed via tile_position);
        # N[b,hpair] = M'-stationary matmuls streaming WpT rows.
        m_drain = {}
        n_drain = {}
        gy = 0

        def m_group(b, hp):
            gm = b * HPAIRS + hp
            slot = gm % 2
            pm = m_ps[slot]
            ochn = (hp * 128) // OCH
            ochn2 = (hp * 128 + 127) // OCH
            slot_wait(m_tenant, slot)
            for kt in range(8):
                tt = b * 8 + kt
                c0 = hp * 128
                need = max(k_drain[(tt, ochn)], v_drain[(tt, ochn)],
                           k_drain[(tt, ochn2)], v_drain[(tt, ochn2)])
                wait("pe", sem_dve, need)
                emit("pe", lambda e, tt=tt, c0=c0, pm=pm, kt=kt: e.matmul(
                    pm[0:D, :], v_sb[:, tt, c0:c0 + D], k_sb[:, tt, c0:c0 + D],
                    start=(kt == 0), stop=(kt == 7), tile_position=(0, 0)))
                mm = lambda e, tt=tt, c0=c0, pm=pm, kt=kt: e.matmul(
                    pm[D:2 * D, :], v_sb[:, tt, c0 + D:c0 + 2 * D],
                    k_sb[:, tt, c0 + D:c0 + 2 * D],
                    start=(kt == 0), stop=(kt == 7), tile_position=(0, 64))
                if kt == 7:
                    emit("pe", lambda e, mm=mm: mm(e).then_inc(sem_pe))
                    cnt["pe"] += 1
                else:
                    emit("pe", mm)
            wait("act", sem_pe, cnt["pe"])
            emit("act", lambda e, gm=gm, pm=pm: e.copy(
                m_sb[0:D, gm, 0:D], pm[0:D, :]).then_inc(sem_act))
            cnt["act"] += 1
            emit("act", lambda e, gm=gm, pm=pm: e.copy(
                m_sb[D:2 * D, gm, D:2 * D], pm[D:2 * D, :]).then_inc(sem_act))
            cnt["act"] += 1
            m_drain[gm] = cnt["act"]
            m_tenant[slot] = ("act", cnt["act"])

        def n_group(b, hp):
            nonlocal gy
            gm = b * HPAIRS + hp
            for och in range(2):
                o0 = och * OCH
                slot = gy % 3
                pn = py_ps[slot]
                wait("pe", sem_act, m_drain[gm])
                wait("pe", sem_wp, 16)
                wait("pe", sem_dve, 2)   # m_sb off-diag zero-fill
                slot_wait(py_tenant, slot)
                # single full-array matmul: block-diag M' contracts both
                # heads of the pair in one 384-col WpT stream
                mm = lambda e, gm=gm, hp=hp, o0=o0, pn=pn: e.matmul(
                    pn[:, 0:OCH], m_sb[:, gm, :],
                    wp_sb[:, hp, o0:o0 + OCH],
                    start=True, stop=True)
                emit("pe", lambda e, mm=mm: mm(e).then_inc(sem_pe))
                cnt["pe"] += 1
                # alternate N drains between ACT and DVE
                dkey = "act" if gy % 2 == 0 else "dve"
                wait(dkey, sem_pe, cnt["pe"])
                if dkey == "act":
                    emit("act", lambda e, gm=gm, o0=o0, pn=pn: e.copy(
                        n_sb[:, gm, o0:o0 + OCH], pn[:, 0:OCH]).then_inc(sem_act))
                else:
                    emit("dve", lambda e, gm=gm, o0=o0, pn=pn: e.tensor_copy(
                        n_sb[:, gm, o0:o0 + OCH], pn[:, 0:OCH]).then_inc(sem_dve))
                cnt[dkey] += 1
                n_drain[(gm, och)] = (dkey, cnt[dkey])
                py_tenant[slot] = (dkey, cnt[dkey])
                gy += 1

        groups = [(b, hp) for b in range(BP) for hp in range(HPAIRS)]
        for i, (b, hp) in enumerate(groups):
            m_group(b, hp)
            if i > 0:
                n_group(*groups[i - 1])
        n_group(*groups[-1])

        # ---- Phase OUT: out[t, o] = qT^T-mm against N_b + bp, DMA out.
        # Stationary = qt token tile (contraction over q channels = hd),
        # moving = N_b rows. One DMA per tile keeps the SP issue rate low;
        # the last tile is split per-chunk so its first half stores while
        # the second half computes.
        slot_dmas = [0] * NOT
        for tt in range(TT):
            b, tch = tt // 8, tt // 4
            slot = tt % NOT
            for och in range(2):
                o0 = och * OCH
                pslot = g % 3
                pz = proj_ps[pslot]
                wait("pe", sem_act,
                     max(qt_drain[(hp2, tch)] for hp2 in range(CT)))
                for dkey in ("act", "dve"):
                    need = max((i for k, i in
                                (n_drain[(b * HPAIRS + hp2, och)]
                                 for hp2 in range(CT))
                                if k == dkey), default=0)
                    wait("pe", ENG_SEM[dkey], need)
                slot_wait(proj_tenant, pslot)
                for hp2 in range(CT):
                    mm = lambda e, hp2=hp2, tt=tt, o0=o0, pz=pz, b=b: e.matmul(
                        pz[:, 0:OCH], qt_sb[:, hp2, tt * 128:(tt + 1) * 128],
                        n_sb[:, b * HPAIRS + hp2, o0:o0 + OCH],
                        start=(hp2 == 0), stop=(hp2 == CT - 1))
                    if hp2 == CT - 1:
                        emit("pe", lambda e, mm=mm: mm(e).then_inc(sem_pe))
                        cnt["pe"] += 1
                    else:
                        emit("pe", mm)
                wait("dve", sem_pe, cnt["pe"])
                if och == 0 and tt >= NOT:
                    wait("dve", sem_out[slot], 16 * slot_dmas[slot])
                emit("dve", lambda e, slot=slot, o0=o0, pz=pz: e.tensor_add(
                    ot_sb[slot][:, o0:o0 + OCH], pz[:, 0:OCH],
                    bp_bc[:, o0:o0 + OCH]).then_inc(sem_dve))
                cnt["dve"] += 1
                g += 1
                if tt == TT - 1:
                    wait("sp", sem_dve, cnt["dve"])
                    emit("sp", lambda e, tt=tt, slot=slot, o0=o0: e.dma_start(
                        out=out_d[tt * 128:(tt + 1) * 128, o0:o0 + OCH],
                        in_=ot_sb[slot][:, o0:o0 + OCH]
                    ).then_inc(sem_out[slot], 16))
                    slot_dmas[slot] += 1
            if tt < TT - 1:
                wait("sp", sem_dve, cnt["dve"])
                emit("sp", lambda e, tt=tt, slot=slot: e.dma_start(
                    out=out_d[tt * 128:(tt + 1) * 128, :], in_=ot_sb[slot][:]
                ).then_inc(sem_out[slot], 16))
                slot_dmas[slot] += 1

        # drain: make sure all output DMAs completed before kernel end
        for s_i in range(NOT):
            wait("sp", sem_out[s_i], 16 * slot_dmas[s_i])

        # ---------------- emit ----------------
        with nc.Block(no_gpsimd_drain=True) as block:

            @block.sync
            def _(e):
                for fn in ops["sp"]:
                    fn(e)

            @block.tensor
            def _(e):
                for fn in ops["pe"]:
                    fn(e)

            @block.scalar
            def _(e):
                for fn in ops["act"]:
                    fn(e)

            @block.vector
            def _(e):
                for fn in ops["dve"]:
                    fn(e)

            @block.gpsimd
            def _(e):
                for fn in ops["pool"]:
                    fn(e)

    return nc


def _get_nc():
    if "nc" not in _CACHE:
        _CACHE["nc"] = _build_nc()
    return _CACHE["nc"]


def _make_in_maps(x, Wq, bq, Wk, bk, Wv, bv, Wp, bp):
    wqT = np.ascontiguousarray(Wq.T).astype(bfloat16)
    wkT = np.ascontiguousarray(Wk.T).astype(bfloat16)
    wvT = np.ascontiguousarray(Wv.T).astype(bfloat16)
    wpT = np.ascontiguousarray(Wp.T).astype(bfloat16)
    # [128, CT] partition-major layout: partition p, column ci holds bq[ci*128+p]
    bqs = (bq * SCALE).astype(np.float32).reshape(CT, 128).T
    biases = np.empty((128, 3 * C), dtype=bfloat16)
    biases[:, 0:C] = np.broadcast_to(bk.astype(bfloat16), (128, C))
    biases[:, C:2 * C] = np.broadcast_to(bv.astype(bfloat16), (128, C))
    biases[:, 2 * C:3 * C] = np.broadcast_to(bp.astype(bfloat16), (128, C))
    # boot[p, a, 0:128] = wqT[a*128+p, 0:128]; boot[p, a, 128:] = xT[a*128+p, 0:512]
    wq_part = wqT[:, 0:128].reshape(CT, 128, 128).transpose(1, 0, 2)
    # wqp[p, co, a, j] = wqT[a*128+p, co*128+j]
    wqp = np.ascontiguousarray(
        wqT.reshape(CT, 128, CT, 128).transpose(1, 2, 0, 3)).astype(bfloat16)
    in_maps = []
    for c in range(N_CORES):
        xs = x[c * BP:(c + 1) * BP].reshape(TOK, C)
        xT = np.ascontiguousarray(xs.T).astype(bfloat16)
        boot = np.empty((128, CT, 128 + QCH + 1), dtype=bfloat16)
        boot[:, :, 0:128] = wq_part
        boot[:, :, 128:128 + QCH] = xT[:, 0:QCH].reshape(CT, 128, QCH).transpose(1, 0, 2)
        boot[:, :, 128 + QCH] = bqs
        in_maps.append({
            "boot": boot, "xT": xT, "wqp": wqp, "wkT": wkT, "wvT": wvT,
            "wpT": wpT, "biases": biases,
        })
    return in_maps


def run(trace=False, tmpdir=None, **inputs):
    from concourse.bass_utils import run_bass_kernel_spmd

    inputs = {k: np.asarray(v, dtype=np.float32) for k, v in inputs.items()}
    nc = _get_nc()
    in_maps = _make_in_maps(**inputs)
    res = run_bass_kernel_spmd(nc, in_maps, core_ids=list(range(N_CORES)),
                               trace=trace, tmpdir=tmpdir)
    out = np.concatenate(
        [res.results[c]["out"].astype(np.float32).reshape(BP, T, C)
         for c in range(N_CORES)], axis=0
    )
    return out, res


def kernel(**inputs):
    out, _ = run(trace=False, **inputs)
    return out

